# revision 1
# baseline (speedup 1.0000x reference)
"""Trainium2 Bass kernel for nn_ExpertRouter (dense MoE routing).

Reference computation (per token t of T=4096, D=6144, MID=512, NE=16):
    h[t,n,:] = relu(xf[t] @ w1[n] + b1[n])          # [T, NE, MID]
    e[t,n]   = h[t,n] . w2[n] + b2[n]               # [T, NE]
    g[t,:]   = softmax(xf[t] @ gw + gb)             # [T, NE]
    out[t]   = sigmoid(sum_n g[t,n] * e[t,n])

Strategy: data-parallel over tokens across 8 NeuronCores (512 tokens/core,
weights replicated, no collectives). Dominant compute = 16 expert matmuls
[512,6144]@[6144,512] per core in bf16 with fp32 PSUM accumulation.
Softmax division is deferred: out = sigmoid((sum_n expl[n]*(e_n+b2)) / sum_n expl[n])
so no per-expert normalization is needed and exp() is computed without
max-subtraction (logit std ~0.58, safe in fp32).

All inputs are rearranged on the host into layouts that make every DMA
fully contiguous per partition.
"""

import numpy as np
import ml_dtypes

# problem constants (hardcoded per harness contract)
B, NW, WS, FD = 16, 256, 8, 96
D = WS * WS * FD          # 6144
MID = 512
NE = 16
T = B * NW                # 4096 tokens
NCORES = 8
TOK = T // NCORES         # 512 tokens per core
P = 128                   # partitions
KT = D // P               # 48 contraction tiles
MT = MID // P             # 4 mid tiles

_CACHE = {}


def _build():
    """Build + compile the per-core SPMD bass program. Returns nc."""
    import concourse.tile as tile
    from concourse import bacc, mybir

    bf16 = mybir.dt.bfloat16
    f32 = mybir.dt.float32
    AF = mybir.ActivationFunctionType
    ALU = mybir.AluOpType

    nc = bacc.Bacc("TRN2", target_bir_lowering=False, debug=False)

    xT_d = nc.dram_tensor("xT", [P, KT, TOK], bf16, kind="ExternalInput").ap()
    w1_d = nc.dram_tensor("w1", [NE, P, KT, MID], bf16, kind="ExternalInput").ap()
    gw_d = nc.dram_tensor("gw", [P, KT, NE], bf16, kind="ExternalInput").ap()
    b1_d = nc.dram_tensor("b1", [P, NE, MT], f32, kind="ExternalInput").ap()
    w2_d = nc.dram_tensor("w2", [P, NE, MT], bf16, kind="ExternalInput").ap()
    b2_d = nc.dram_tensor("b2", [1, NE], f32, kind="ExternalInput").ap()
    gb_d = nc.dram_tensor("gb", [NE, 1], f32, kind="ExternalInput").ap()
    out_d = nc.dram_tensor("out", [1, TOK], f32, kind="ExternalOutput").ap()

    with tile.TileContext(nc) as tc:
        with (
            tc.tile_pool(name="consts", bufs=1) as consts,
            tc.tile_pool(name="xpool", bufs=1) as xpool,
            tc.tile_pool(name="wpool", bufs=2) as wpool,
            tc.tile_pool(name="hpool", bufs=3) as hpool,
            tc.tile_pool(name="small", bufs=2) as small,
            tc.tile_pool(name="acc", bufs=1) as accp,
            tc.tile_pool(name="ps_h", bufs=2, space="PSUM") as ps_h,
            tc.tile_pool(name="ps_g", bufs=1, space="PSUM") as ps_g,
            tc.tile_pool(name="ps_e", bufs=2, space="PSUM") as ps_e,
        ):
            # resident inputs
            xT = xpool.tile([P, KT, TOK], bf16)
            nc.sync.dma_start(xT[:], xT_d[:])
            gw = consts.tile([P, KT, NE], bf16)
            nc.sync.dma_start(gw[:], gw_d[:])
            b1 = consts.tile([P, NE, MT], f32)
            nc.sync.dma_start(b1[:], b1_d[:])
            w2 = consts.tile([P, NE, MT], bf16)
            nc.sync.dma_start(w2[:], w2_d[:])
            b2 = consts.tile([1, NE], f32)
            nc.sync.dma_start(b2[:], b2_d[:])
            gb = consts.tile([NE, 1], f32)
            nc.sync.dma_start(gb[:], gb_d[:])
            ones = consts.tile([NE, 1], f32)
            nc.vector.memset(ones[:], 1.0)

            # gating logits: gl[e, t] = sum_d gw[d, e] * x[d, t]
            gl = ps_g.tile([NE, TOK], f32)
            for k in range(KT):
                nc.tensor.matmul(
                    gl[:], gw[:, k, :], xT[:, k, :], start=(k == 0), stop=(k == KT - 1)
                )
            expl = consts.tile([NE, TOK], f32)
            nc.scalar.activation(expl[:], gl[:], AF.Exp, bias=gb[:])

            # denominator: den[t] = sum_e expl[e, t]
            den = ps_g.tile([1, TOK], f32)
            nc.tensor.matmul(den[:], ones[:], expl[:], start=True, stop=True)
            rec = consts.tile([1, TOK], f32)
            nc.vector.reciprocal(rec[:], den[:])

            # flatten expl rows onto partition 0 so per-expert weighting is
            # a partition-0 elementwise op (cross-partition move via DMA)
            explf = consts.tile([1, NE * TOK], f32)
            for n in range(NE):
                nc.sync.dma_start(explf[0:1, n * TOK:(n + 1) * TOK], expl[n:n + 1, :])

            # weighted-sum accumulator on partition 0
            u = accp.tile([1, TOK], f32)
            nc.vector.memset(u[:], 0.0)

            for n in range(NE):
                w1s = wpool.tile([P, KT, MID], bf16)
                nc.sync.dma_start(w1s[:], w1_d[n])
                e_ps = ps_e.tile([1, TOK], f32)
                for mt in range(MT):
                    h_ps = ps_h.tile([P, TOK], f32)
                    for k in range(KT):
                        nc.tensor.matmul(
                            h_ps[:],
                            w1s[:, k, mt * P:(mt + 1) * P],
                            xT[:, k, :],
                            start=(k == 0),
                            stop=(k == KT - 1),
                        )
                    h_sb = hpool.tile([P, TOK], bf16)
                    nc.scalar.activation(
                        h_sb[:], h_ps[:], AF.Relu, bias=b1[:, n, mt:mt + 1]
                    )
                    nc.tensor.matmul(
                        e_ps[:],
                        w2[:, n, mt:mt + 1],
                        h_sb[:],
                        start=(mt == 0),
                        stop=(mt == MT - 1),
                        skip_group_check=True,
                    )
                # u += (e + b2[n]) * expl[n]
                tmp = small.tile([1, TOK], f32)
                nc.vector.scalar_tensor_tensor(
                    tmp[:],
                    e_ps[:],
                    b2[0:1, n:n + 1],
                    explf[0:1, n * TOK:(n + 1) * TOK],
                    ALU.add,
                    ALU.mult,
                )
                nc.vector.tensor_add(u[:], u[:], tmp[:])

            # out = sigmoid(u / den)
            s = small.tile([1, TOK], f32)
            nc.vector.tensor_mul(s[:], u[:], rec[:])
            o = small.tile([1, TOK], f32)
            nc.scalar.activation(o[:], s[:], AF.Sigmoid)
            nc.sync.dma_start(out_d[:], o[:])

    nc.compile()
    return nc


def _prep_inputs(x, w1, b1, w2, b2, gw, gb):
    """Host-side shard + layout prep. Returns per-core in_maps."""
    bf = ml_dtypes.bfloat16
    xf = np.ascontiguousarray(np.asarray(x, np.float32)).reshape(T, D)
    # xT[core][p, k, t] = xf[core*TOK + t, k*P + p]
    xp = xf.reshape(NCORES, TOK, KT, P).transpose(0, 3, 2, 1).astype(bf)
    w1p = (
        np.asarray(w1, np.float32)
        .reshape(NE, KT, P, MID)
        .transpose(0, 2, 1, 3)
        .astype(bf)
    )
    w1p = np.ascontiguousarray(w1p)
    gwp = np.ascontiguousarray(
        np.asarray(gw, np.float32).reshape(KT, P, NE).transpose(1, 0, 2).astype(bf)
    )
    b1p = np.ascontiguousarray(
        np.asarray(b1, np.float32).reshape(NE, MT, P).transpose(2, 0, 1)
    )
    w2p = np.ascontiguousarray(
        np.asarray(w2, np.float32).reshape(NE, MT, P).transpose(2, 0, 1).astype(bf)
    )
    b2p = np.asarray(b2, np.float32).reshape(1, NE)
    gbp = np.asarray(gb, np.float32).reshape(NE, 1)

    in_maps = []
    for c in range(NCORES):
        in_maps.append(
            {
                "xT": np.ascontiguousarray(xp[c]),
                "w1": w1p,
                "gw": gwp,
                "b1": b1p,
                "w2": w2p,
                "b2": b2p,
                "gb": gbp,
            }
        )
    return in_maps


def kernel(x, w1, b1, w2, b2, gw, gb):
    from concourse import bass_utils

    if "nc" not in _CACHE:
        _CACHE["nc"] = _build()
    nc = _CACHE["nc"]
    in_maps = _prep_inputs(x, w1, b1, w2, b2, gw, gb)
    res = bass_utils.run_bass_kernel_spmd(nc, in_maps, core_ids=list(range(NCORES)))
    out = np.concatenate([r["out"].reshape(TOK) for r in res.results])
    return out.reshape(B, NW).astype(np.float32)


# revision 13
# speedup vs baseline: 93.6014x; 93.6014x over previous
"""Trainium2 Bass kernel for nn_ExpertRouter (dense MoE routing).

Reference computation (per token t of T=4096, D=6144, MID=512, NE=16):
    h[t,n,:] = relu(xf[t] @ w1[n] + b1[n])          # [T, NE, MID]
    e[t,n]   = h[t,n] . w2[n] + b2[n]               # [T, NE]
    g[t,:]   = softmax(xf[t] @ gw + gb)             # [T, NE]
    out[t]   = sigmoid(sum_n g[t,n] * e[t,n])

Strategy: data-parallel over tokens across 8 NeuronCores (512 tokens/core,
weights replicated, no collectives). Dominant compute = 16 expert matmuls
[512,6144]@[6144,512] per core in bf16 with fp32 PSUM accumulation.
Softmax division is deferred: out = sigmoid((sum_n expl[n]*(e_n+b2)) / sum_n expl[n])
so no per-expert normalization is needed and exp() is computed without
max-subtraction (logit std ~0.58, safe in fp32).

All inputs are rearranged on the host into layouts that make every DMA
fully contiguous per partition.
"""

import numpy as np
import ml_dtypes

# problem constants (hardcoded per harness contract)
B, NW, WS, FD = 16, 256, 8, 96
D = WS * WS * FD          # 6144
MID = 512
NE = 16
T = B * NW                # 4096 tokens
NCORES = 8
TOK = T // NCORES         # 512 tokens per core
P = 128                   # partitions
KT = D // P               # 48 contraction tiles
MT = MID // P             # 4 mid tiles

_CACHE = {}


import os

USE_FP8 = os.environ.get("ER_FP8", "0") == "1"  # fp8-e4m3 DoubleRow big matmuls
FP8_SCALE = 128.0  # w1 pre-scale so U(-1/sqrt(D),..) lands in e4m3 normal range
KT2 = KT // 2      # DoubleRow k-steps (2 contraction rows per partition)


def _build(reps=1, use_fp8=USE_FP8):
    """Build + compile the per-core SPMD bass program. Returns nc.

    reps>1 wraps the whole body in a Tile For loop — used only for
    slope-based HW timing (fixed dispatch overhead cancels between rep
    counts); the graded kernel uses reps=1 (no loop)."""
    import contextlib
    import concourse.tile as tile
    from concourse import bacc, mybir

    bf16 = mybir.dt.bfloat16
    fp8 = mybir.dt.float8e4
    f32 = mybir.dt.float32
    AF = mybir.ActivationFunctionType
    ALU = mybir.AluOpType

    nc = bacc.Bacc("TRN2", target_bir_lowering=False, debug=False)

    xT_d = nc.dram_tensor("xT", [P, KT, TOK], bf16, kind="ExternalInput").ap()
    if use_fp8:
        xq_d = nc.dram_tensor("xq", [P, KT2, 2, TOK], fp8, kind="ExternalInput").ap()
        w1_d = nc.dram_tensor(
            "w1", [NE, P, KT2, 2, MID], fp8, kind="ExternalInput"
        ).ap()
    else:
        # mt-major layout so each [P, KT, 128] mid-chunk is one contiguous DMA
        w1_d = nc.dram_tensor(
            "w1", [NE, MT, P, KT, P], bf16, kind="ExternalInput"
        ).ap()
    gw_d = nc.dram_tensor("gw", [P, KT, NE], bf16, kind="ExternalInput").ap()
    b1_d = nc.dram_tensor("b1", [P, NE, MT], f32, kind="ExternalInput").ap()
    w2_d = nc.dram_tensor("w2", [P, NE, MT], bf16, kind="ExternalInput").ap()
    b2_d = nc.dram_tensor("b2", [1, NE], f32, kind="ExternalInput").ap()
    gb_d = nc.dram_tensor("gb", [NE, 1], f32, kind="ExternalInput").ap()
    out_d = nc.dram_tensor("out", [1, TOK], f32, kind="ExternalOutput").ap()

    with tile.TileContext(nc) as tc:
        loop_ctx = (
            tc.For_i(0, reps, 1) if reps > 1 else contextlib.nullcontext()
        )
        with (
            loop_ctx,
            tc.tile_pool(name="consts", bufs=1) as consts,
            tc.tile_pool(name="xpool", bufs=1) as xpool,
            tc.tile_pool(name="wpool", bufs=2) as wpool,
            tc.tile_pool(name="hpool", bufs=3) as hpool,
            tc.tile_pool(name="small", bufs=2) as small,
            tc.tile_pool(name="acc", bufs=1) as accp,
            tc.tile_pool(name="ps_h", bufs=2, space="PSUM") as ps_h,
            tc.tile_pool(name="ps_g", bufs=1, space="PSUM") as ps_g,
            tc.tile_pool(name="ps_e", bufs=2, space="PSUM") as ps_e,
        ):
            # resident inputs; xT DMA'd in 8 k-chunks so gating (and the
            # first expert) can start as soon as early chunks land
            XCH = 6
            xT = xpool.tile([P, KT, TOK], bf16)
            for c in range(KT // XCH):
                nc.sync.dma_start(
                    xT[:, c * XCH:(c + 1) * XCH, :], xT_d[:, c * XCH:(c + 1) * XCH, :]
                )
            if use_fp8:
                xq = xpool.tile([P, KT2, 2, TOK], fp8)
                nc.sync.dma_start(xq[:], xq_d[:])
            gw = consts.tile([P, KT, NE], bf16)
            nc.sync.dma_start(gw[:], gw_d[:])
            b1 = consts.tile([P, NE, MT], f32)
            nc.sync.dma_start(b1[:], b1_d[:])
            w2 = consts.tile([P, NE, MT], bf16)
            nc.sync.dma_start(w2[:], w2_d[:])
            b2 = consts.tile([1, NE], f32)
            nc.sync.dma_start(b2[:], b2_d[:])
            gb = consts.tile([NE, 1], f32)
            nc.sync.dma_start(gb[:], gb_d[:])
            ones = consts.tile([NE, 1], f32)
            nc.vector.memset(ones[:], 1.0)

            # gating logits: gl[e, t] = sum_d gw[d, e] * x[d, t]
            gl = ps_g.tile([NE, TOK], f32)
            for k in range(KT):
                nc.tensor.matmul(
                    gl[:], gw[:, k, :], xT[:, k, :], start=(k == 0), stop=(k == KT - 1)
                )
            expl = consts.tile([NE, TOK], f32)
            nc.scalar.activation(expl[:], gl[:], AF.Exp, bias=gb[:])

            # denominator: den[t] = sum_e expl[e, t]
            den = ps_g.tile([1, TOK], f32)
            nc.tensor.matmul(den[:], ones[:], expl[:], start=True, stop=True)
            rec = consts.tile([1, TOK], f32)
            nc.vector.reciprocal(rec[:], den[:])

            # flatten expl rows onto partition 0 so per-expert weighting is
            # a partition-0 elementwise op (cross-partition move via DMA)
            explf = consts.tile([1, NE * TOK], f32)
            for n in range(NE):
                nc.sync.dma_start(explf[0:1, n * TOK:(n + 1) * TOK], expl[n:n + 1, :])

            # weighted-sum accumulator on partition 0
            u = accp.tile([1, TOK], f32)
            nc.vector.memset(u[:], 0.0)

            for n in range(NE):
                if use_fp8:
                    w1s = wpool.tile([P, KT2, 2, MID], fp8)
                    nc.sync.dma_start(w1s[:], w1_d[n])
                e_ps = ps_e.tile([1, TOK], f32)
                for mt in range(MT):
                    if not use_fp8:
                        # one [P, KT, 128] weight chunk per mid-tile: finer
                        # prefetch granularity, smaller SBUF footprint
                        w1c = wpool.tile([P, KT, P], bf16, bufs=6)
                        nc.sync.dma_start(w1c[:], w1_d[n, mt])
                    h_ps = ps_h.tile([P, TOK], f32)
                    if use_fp8:
                        for k2 in range(KT2):
                            nc.tensor.matmul(
                                h_ps[:],
                                w1s[:, k2, :, mt * P:(mt + 1) * P],
                                xq[:, k2, :, :],
                                start=(k2 == 0),
                                stop=(k2 == KT2 - 1),
                                perf_mode=mybir.MatmulPerfMode.DoubleRow,
                            )
                    else:
                        for k in range(KT):
                            nc.tensor.matmul(
                                h_ps[:],
                                w1c[:, k, :],
                                xT[:, k, :],
                                start=(k == 0),
                                stop=(k == KT - 1),
                            )
                    h_sb = hpool.tile([P, TOK], bf16)
                    nc.scalar.activation(
                        h_sb[:],
                        h_ps[:],
                        AF.Relu,
                        bias=b1[:, n, mt:mt + 1],
                        scale=(1.0 / FP8_SCALE) if use_fp8 else 1.0,
                    )
                    nc.tensor.matmul(
                        e_ps[:],
                        w2[:, n, mt:mt + 1],
                        h_sb[:],
                        start=(mt == 0),
                        stop=(mt == MT - 1),
                        skip_group_check=True,
                    )
                # u += (e + b2[n]) * expl[n]
                tmp = small.tile([1, TOK], f32)
                nc.vector.scalar_tensor_tensor(
                    tmp[:],
                    e_ps[:],
                    b2[0:1, n:n + 1],
                    explf[0:1, n * TOK:(n + 1) * TOK],
                    ALU.add,
                    ALU.mult,
                )
                nc.vector.tensor_add(u[:], u[:], tmp[:])

            # out = sigmoid(u / den)
            s = small.tile([1, TOK], f32)
            nc.vector.tensor_mul(s[:], u[:], rec[:])
            o = small.tile([1, TOK], f32)
            nc.scalar.activation(o[:], s[:], AF.Sigmoid)
            nc.sync.dma_start(out_d[:], o[:])

    nc.compile()
    return nc


def _prep_inputs(x, w1, b1, w2, b2, gw, gb, use_fp8=USE_FP8):
    """Host-side shard + layout prep. Returns per-core in_maps."""
    bf = ml_dtypes.bfloat16
    import concourse.mybir as mybir

    fp8np = mybir.dt.np(mybir.dt.float8e4)
    xf = np.ascontiguousarray(np.asarray(x, np.float32)).reshape(T, D)
    # xT[core][p, k, t] = xf[core*TOK + t, k*P + p]
    xp = xf.reshape(NCORES, TOK, KT, P).transpose(0, 3, 2, 1).astype(bf)
    if use_fp8:
        # xq[core][p, k2, h, t] = xf[core*TOK + t, k2*256 + h*128 + p]
        xqp = (
            xf.reshape(NCORES, TOK, KT2, 2, P)
            .transpose(0, 4, 2, 3, 1)
            .astype(fp8np)
        )
        # w1q[n, p, k2, h, m] = FP8_SCALE * w1[n, k2*256 + h*128 + p, m]
        w1p = np.ascontiguousarray(
            (np.asarray(w1, np.float32) * FP8_SCALE)
            .reshape(NE, KT2, 2, P, MID)
            .transpose(0, 3, 1, 2, 4)
            .astype(fp8np)
        )
    else:
        # [NE, MT, P, KT, P]: w1p[n, mt, p, k, m] = w1[n, k*P+p, mt*P+m]
        w1p = np.ascontiguousarray(
            np.asarray(w1, np.float32)
            .reshape(NE, KT, P, MT, P)
            .transpose(0, 3, 2, 1, 4)
            .astype(bf)
        )
    gwp = np.ascontiguousarray(
        np.asarray(gw, np.float32).reshape(KT, P, NE).transpose(1, 0, 2).astype(bf)
    )
    b1p = np.ascontiguousarray(
        np.asarray(b1, np.float32).reshape(NE, MT, P).transpose(2, 0, 1)
    )
    w2p = np.ascontiguousarray(
        np.asarray(w2, np.float32).reshape(NE, MT, P).transpose(2, 0, 1).astype(bf)
    )
    b2p = np.asarray(b2, np.float32).reshape(1, NE)
    gbp = np.asarray(gb, np.float32).reshape(NE, 1)

    in_maps = []
    for c in range(NCORES):
        m = {
            "xT": np.ascontiguousarray(xp[c]),
            "w1": w1p,
            "gw": gwp,
            "b1": b1p,
            "w2": w2p,
            "b2": b2p,
            "gb": gbp,
        }
        if use_fp8:
            m["xq"] = np.ascontiguousarray(xqp[c])
        in_maps.append(m)
    return in_maps


def kernel(x, w1, b1, w2, b2, gw, gb):
    from concourse import bass_utils

    if "nc" not in _CACHE:
        _CACHE["nc"] = _build()
    nc = _CACHE["nc"]
    in_maps = _prep_inputs(x, w1, b1, w2, b2, gw, gb)
    res = bass_utils.run_bass_kernel_spmd(nc, in_maps, core_ids=list(range(NCORES)))
    out = np.concatenate([r["out"].reshape(TOK) for r in res.results])
    return out.reshape(B, NW).astype(np.float32)
